# revision 8
# baseline (speedup 1.0000x reference)
"""Trainium2 Bass kernel for the scatter_memory problem.

Math (reference):
    scores[b,m,u] = sum_d attention[b,u,d] * mem_attention[m,u,d]
    scores = where(mask, -1e30, scores) / temperature[u]
    weights = softmax(scores, axis=m)                    # [B, M, U]
    outputs[b,u,d] = sum_m weights[b,m,u] * memory[m,u,d]
    returns (outputs, weights, memory)

Sharding: over the unit axis U (64 units -> 8 units per core). Softmax is
over M, which stays local, so there are no collectives. Each core reads
only its slice of mem_attention / memory / mask, writes its slice of
weights -- the big memory-bank tensors are read exactly once chip-wide.

Per-core device program (8 units, pipelined by the Tile scheduler):
  mm1:   scores[b, m]  = attT(u)^T @ mem_attT(u)   (contraction d=32)
  mask:  copy_predicated(-1e30 where mask) into PSUM
  exp:   ScalarE exp PSUM->SBUF with accum_out giving row sums
  norm:  w = em * (1/sum)  (VectorE tensor_scalar, per-partition scalar)
  mm2:   transpose w tiles on TensorE, then out[d,b] += mem_tile^T @ wT_tile
Temperature is folded into attT on the host (scores/t == (att/t).mem_att).
"""

import threading
from contextlib import ExitStack

import numpy as np

import concourse.bass as bass
import concourse.mybir as mybir
import concourse.tile as tile
from concourse import bacc
from concourse.bass_utils import run_bass_kernel_spmd
from concourse.masks import make_identity

B, M, U, D = 128, 4096, 64, 32
NCORES = 8
UPC = U // NCORES  # units per core = 8
MC = 1024          # m-chunk size (2 PSUM banks)
NCHUNK = M // MC   # 4
NTILE = MC // 128  # 8 m-tiles of 128 per chunk
NEG = -1e30
F32 = mybir.dt.float32
U8 = mybir.dt.uint8

_BUILD_LOCK = threading.Lock()
_NC_CACHE = {}


def _build_nc():
    nc = bacc.Bacc("TRN2", target_bir_lowering=False)

    att_t = nc.declare_dram_parameter("att_t", [D, UPC * B], F32, isOutput=False)
    mem_at = nc.declare_dram_parameter("mem_att_t", [UPC, D, M], F32, isOutput=False)
    mask8 = nc.declare_dram_parameter("mask_u8", [UPC, B, M], U8, isOutput=False)
    membk = nc.declare_dram_parameter(
        "mem_bank", [UPC, 128, M // 128, D], F32, isOutput=False
    )
    w_out = nc.declare_dram_parameter("w_out", [UPC, B, M], F32, isOutput=True)
    o_out = nc.declare_dram_parameter("o_out", [D, UPC * B], F32, isOutput=True)

    AF = mybir.ActivationFunctionType

    with ExitStack() as ctx:
        tc = ctx.enter_context(tile.TileContext(nc))

        const = ctx.enter_context(tc.tile_pool(name="const", bufs=1))
        ma_pool = ctx.enter_context(tc.tile_pool(name="ma", bufs=2))
        mk_pool = ctx.enter_context(tc.tile_pool(name="mk", bufs=2))
        mem_pool = ctx.enter_context(tc.tile_pool(name="mem", bufs=2))
        em_pool = ctx.enter_context(tc.tile_pool(name="em", bufs=2))
        w_pool = ctx.enter_context(tc.tile_pool(name="w", bufs=2))
        et_pool = ctx.enter_context(tc.tile_pool(name="et", bufs=2))
        stat_pool = ctx.enter_context(tc.tile_pool(name="stat", bufs=2))
        osb_pool = ctx.enter_context(tc.tile_pool(name="osb", bufs=1))

        sp_pool = ctx.enter_context(tc.tile_pool(name="sp", bufs=2, space="PSUM"))
        tp_pool = ctx.enter_context(tc.tile_pool(name="tp", bufs=1, space="PSUM"))
        op_pool = ctx.enter_context(tc.tile_pool(name="op", bufs=2, space="PSUM"))

        ident = const.tile([128, 128], F32)
        make_identity(nc, ident[:])
        neginf = const.tile([128, MC], F32)
        nc.gpsimd.memset(neginf[:], NEG)
        atts = const.tile([D, UPC * B], F32)
        nc.sync.dma_start(atts[:], att_t[:])
        obuf = osb_pool.tile([D, UPC * B], F32)

        for u in range(UPC):
            ma = ma_pool.tile([D, M], F32)
            nc.sync.dma_start(ma[:], mem_at[u])
            mk = mk_pool.tile([B, M], U8)
            nc.sync.dma_start(mk[:], mask8[u])
            mem = mem_pool.tile([128, (M // 128) * D], F32)
            nc.sync.dma_start(mem[:], membk[u].rearrange("p t d -> p (t d)"))

            em = em_pool.tile([B, M], F32)
            sums = stat_pool.tile([B, 8], F32)

            for c in range(NCHUNK):
                sp = sp_pool.tile([B, MC], F32)
                for h in range(MC // 512):
                    nc.tensor.matmul(
                        sp[:, h * 512 : (h + 1) * 512],
                        atts[:, u * B : (u + 1) * B],
                        ma[:, c * MC + h * 512 : c * MC + (h + 1) * 512],
                        start=True,
                        stop=True,
                    )
                nc.vector.copy_predicated(
                    sp[:, :], mk[:, c * MC : (c + 1) * MC], neginf[:, :]
                )
                nc.scalar.activation(
                    em[:, c * MC : (c + 1) * MC],
                    sp[:, :],
                    AF.Exp,
                    accum_out=sums[:, c : c + 1],
                )

            nc.vector.tensor_reduce(
                sums[:, 4:5],
                sums[:, 0:NCHUNK],
                axis=mybir.AxisListType.X,
                op=mybir.AluOpType.add,
            )
            nc.vector.reciprocal(sums[:, 5:6], sums[:, 4:5])

            w = w_pool.tile([B, M], F32)
            nc.vector.tensor_scalar_mul(w[:], em[:], sums[:, 5:6])
            nc.sync.dma_start(w_out[u], w[:])

            ot = op_pool.tile([D, B], F32)
            for c in range(NCHUNK):
                tp = tp_pool.tile([B, MC], F32)
                for t in range(NTILE):
                    nc.tensor.transpose(
                        tp[:, t * 128 : (t + 1) * 128],
                        w[:, c * MC + t * 128 : c * MC + (t + 1) * 128],
                        ident[:],
                    )
                et = et_pool.tile([B, MC], F32)
                nc.scalar.copy(et[:], tp[:])
                for t in range(NTILE):
                    mt = c * NTILE + t
                    nc.tensor.matmul(
                        ot[:],
                        mem[:, mt * D : (mt + 1) * D],
                        et[:, t * 128 : (t + 1) * 128],
                        start=(mt == 0),
                        stop=(mt == M // 128 - 1),
                    )
            nc.scalar.copy(obuf[:, u * B : (u + 1) * B], ot[:])

        nc.sync.dma_start(o_out[:], obuf[:])

    nc.compile()
    return nc


def _get_nc():
    with _BUILD_LOCK:
        if "nc" not in _NC_CACHE:
            _NC_CACHE["nc"] = _build_nc()
        return _NC_CACHE["nc"]


def _prep_inputs(attention, mem_attention, memory, temperature, mask):
    """Host-side resharding to the per-core device layouts."""
    attention = np.asarray(attention, np.float32)
    mem_attention = np.asarray(mem_attention, np.float32)
    memory = np.asarray(memory, np.float32)
    temperature = np.asarray(temperature, np.float32)
    mask = np.asarray(mask)

    att = attention / temperature[None, :, None]  # fold temperature into scores
    attT = att.transpose(1, 2, 0)  # [U, D, B]
    maT = mem_attention.transpose(1, 2, 0)  # [U, D, M]
    mkT = np.ascontiguousarray(mask.transpose(2, 0, 1)).view(np.uint8)  # [U, B, M]
    # memory [M, U, D] -> [U, 128(p), M//128(t), D] with m = t*128 + p
    memT = memory.transpose(1, 0, 2).reshape(U, M // 128, 128, D).transpose(0, 2, 1, 3)

    in_maps = []
    for c in range(NCORES):
        sl = slice(c * UPC, (c + 1) * UPC)
        in_maps.append(
            {
                "att_t": np.ascontiguousarray(
                    attT[sl].transpose(1, 0, 2).reshape(D, UPC * B)
                ),
                "mem_att_t": np.ascontiguousarray(maT[sl]),
                "mask_u8": np.ascontiguousarray(mkT[sl]),
                "mem_bank": np.ascontiguousarray(memT[sl]),
            }
        )
    return in_maps, memory


def _assemble(results):
    weights = np.empty((B, M, U), np.float32)
    outputs = np.empty((B, U, D), np.float32)
    for c in range(NCORES):
        w_core = results[c]["w_out"]  # [UPC, B, M]
        weights[:, :, c * UPC : (c + 1) * UPC] = w_core.transpose(1, 2, 0)
        o_core = results[c]["o_out"].reshape(D, UPC, B)  # [D, u, b]
        outputs[:, c * UPC : (c + 1) * UPC, :] = o_core.transpose(2, 1, 0)
    return outputs, weights


def run(attention, mem_attention, memory, temperature, mask, trace=False, **trace_kwargs):
    """Run on the 8 NeuronCores; returns ((outputs, weights, memory), BassKernelResults)."""
    in_maps, memory_np = _prep_inputs(
        attention, mem_attention, memory, temperature, mask
    )
    nc = _get_nc()
    res = run_bass_kernel_spmd(
        nc, in_maps, list(range(NCORES)), trace=trace, **trace_kwargs
    )
    outputs, weights = _assemble(res.results)
    return (outputs, weights, memory_np), res


def kernel(attention, mem_attention, memory, temperature, mask):
    out, _ = run(attention, mem_attention, memory, temperature, mask, trace=False)
    return out
